# revision 1
# baseline (speedup 1.0000x reference)
"""Fused LSTM cell on 8 Trainium2 NeuronCores.

Data-parallel over the batch: each core handles 1024 of the 8192 rows.
Per core, the two GEMMs (x @ Wx.T + h @ Wh.T) are fused into one
[2048]-contraction GEMM in float32r (full-rate fp32 PE mode), with the
gate nonlinearities + state update fused into the PSUM eviction path.

Layouts are chosen so every DMA is a plain strided copy (no on-chip
transposes): activations and weights are pre-transposed on the host so
the contraction dim lands on SBUF partitions, and the whole kernel runs
in [hidden, batch] layout; the host transposes the outputs back.
"""

import os

import numpy as np

import concourse.bacc as bacc
import concourse.mybir as mybir
import concourse.tile as tile
from concourse.bass_utils import run_bass_kernel_spmd

B, I, H = 8192, 1024, 1024
NCORES = 8
BL = B // NCORES        # batch rows per core
G4 = 4 * H              # stacked gate dim
KC = (I + H) // 128     # contraction chunks of 128
HT = H // 128           # h-tiles per core
NBC = 2                 # batch chunks per h-tile
BCW = BL // NBC         # 512 columns per matmul (one PSUM bank)

F32 = mybir.dt.float32
F32R = mybir.dt.float32r
AF = mybir.ActivationFunctionType
OP = mybir.AluOpType

_CACHE: dict = {}


def _build(reps=1, hw_loop=False, variant="full"):
    mm_dt = mybir.dt.bfloat16 if variant == "mm_bf16" else F32R
    nc = bacc.Bacc("TRN2", target_bir_lowering=False, debug=False)
    aT = nc.dram_tensor("a_t", [I + H, BL], mm_dt, kind="ExternalInput")
    wT = nc.dram_tensor("w_t", [I + H, G4], mm_dt, kind="ExternalInput")
    cT = nc.dram_tensor("c_t", [H, BL], F32, kind="ExternalInput")
    bias = nc.dram_tensor("bias", [128, 4 * HT], F32, kind="ExternalInput")
    cO = nc.dram_tensor("c_out", [H, BL], F32, kind="ExternalOutput")
    hO = nc.dram_tensor("h_out", [H, BL], F32, kind="ExternalOutput")

    with tile.TileContext(nc) as tc:
        with (
            tc.tile_pool(name="resident", bufs=1) as res_pool,
            tc.tile_pool(name="wpool", bufs=2) as w_pool,
            tc.tile_pool(name="cpool", bufs=2) as c_pool,
            tc.tile_pool(name="opool", bufs=2) as o_pool,
            tc.tile_pool(name="act", bufs=3) as act_pool,
            tc.tile_pool(name="psum", bufs=2, space="PSUM") as psum_pool,
        ):
            # Activations resident for the whole kernel: [128, 16, 1024]
            a_sb = res_pool.tile([128, KC, BL], mm_dt)
            nc.sync.dma_start(a_sb[:], aT.rearrange("(c p) b -> p c b", p=128))
            bias_sb = res_pool.tile([128, 4 * HT], F32)
            nc.sync.dma_start(bias_sb[:], bias[:])

            # [p, kchunk, gate, htile, col]
            w_r = wT.rearrange("(c p) (G t g) -> p c G t g", p=128, G=4, g=128)

            w0_sb = None
            if variant == "mm_nodma":
                w0_sb = res_pool.tile([128, KC, 4, 128], mm_dt)
                for g in range(4):
                    nc.sync.dma_start(w0_sb[:, :, g, :], w_r[:, :, g, 0, :])

            def body_mm_only():
                # PE + weight-DMA path only: no epilogue, no outputs.
                for t in range(HT):
                    if variant == "mm_nodma":
                        w_sb = w0_sb
                    else:
                        w_sb = w_pool.tile([128, KC, 4, 128], mm_dt, tag="w",
                                           name="w_sb")
                        for g in range(4):
                            nc.sync.dma_start(w_sb[:, :, g, :], w_r[:, :, g, t, :])
                    for bc in range(NBC):
                        bsl = slice(bc * BCW, (bc + 1) * BCW)
                        for g in range(4):
                            p_t = psum_pool.tile([128, BCW], F32, tag=f"ps{g}")
                            for c in range(KC):
                                nc.tensor.matmul(
                                    p_t[:], w_sb[:, c, g, :], a_sb[:, c, bsl],
                                    start=(c == 0), stop=(c == KC - 1),
                                )

            def body_ldw_reuse():
                # Same math as "full", but each stationary tile feeds the two
                # batch-chunk matmuls back to back (bc innermost).
                for t in range(HT):
                    w_sb = w_pool.tile([128, KC, 4, 128], F32R, tag="w")
                    for g in range(4):
                        nc.sync.dma_start(w_sb[:, :, g, :], w_r[:, :, g, t, :])
                    cp_sb = c_pool.tile([128, BL], F32, tag="cprev")
                    nc.sync.dma_start(cp_sb[:], cT[t * 128:(t + 1) * 128, :])
                    oc_sb = o_pool.tile([128, BL], F32, tag="oc")
                    oh_sb = o_pool.tile([128, BL], F32, tag="oh")

                    ps = {}
                    for g in range(4):
                        for bc in range(NBC):
                            ps[g, bc] = psum_pool.tile(
                                [128, BCW], F32, tag=f"ps{g}{bc}",
                                name=f"ps{g}{bc}", bufs=1)
                        for c in range(KC):
                            for bc in range(NBC):
                                bsl = slice(bc * BCW, (bc + 1) * BCW)
                                nc.tensor.matmul(
                                    ps[g, bc][:], w_sb[:, c, g, :], a_sb[:, c, bsl],
                                    start=(c == 0), stop=(c == KC - 1),
                                )
                    for bc in range(NBC):
                        bsl = slice(bc * BCW, (bc + 1) * BCW)
                        si = act_pool.tile([128, BCW], F32, tag="si")
                        sf = act_pool.tile([128, BCW], F32, tag="sf")
                        so = act_pool.tile([128, BCW], F32, tag="so")
                        tg = act_pool.tile([128, BCW], F32, tag="tg")
                        nc.scalar.activation(si[:], ps[0, bc][:], AF.Sigmoid,
                                             bias=bias_sb[:, 0 * HT + t:0 * HT + t + 1])
                        nc.scalar.activation(sf[:], ps[1, bc][:], AF.Sigmoid,
                                             bias=bias_sb[:, 1 * HT + t:1 * HT + t + 1])
                        nc.scalar.activation(so[:], ps[2, bc][:], AF.Sigmoid,
                                             bias=bias_sb[:, 2 * HT + t:2 * HT + t + 1])
                        nc.scalar.activation(tg[:], ps[3, bc][:], AF.Tanh,
                                             bias=bias_sb[:, 3 * HT + t:3 * HT + t + 1])
                        t1 = act_pool.tile([128, BCW], F32, tag="t1")
                        t2 = act_pool.tile([128, BCW], F32, tag="t2")
                        nc.vector.tensor_tensor(t1[:], sf[:], cp_sb[:, bsl], OP.mult)
                        nc.vector.tensor_tensor(t2[:], si[:], tg[:], OP.mult)
                        nc.vector.tensor_tensor(oc_sb[:, bsl], t1[:], t2[:], OP.add)
                        tct = act_pool.tile([128, BCW], F32, tag="tct")
                        nc.scalar.activation(tct[:], oc_sb[:, bsl], AF.Tanh)
                        nc.vector.tensor_tensor(oh_sb[:, bsl], so[:], tct[:], OP.mult)

                    nc.sync.dma_start(cO[t * 128:(t + 1) * 128, :], oc_sb[:])
                    nc.sync.dma_start(hO[t * 128:(t + 1) * 128, :], oh_sb[:])

            def body():
                if variant in ("mm_only", "mm_nodma", "mm_bf16"):
                    return body_mm_only()
                if variant == "ldw_reuse":
                    return body_ldw_reuse()
                for t in range(HT):
                    w_sb = w_pool.tile([128, KC, 4, 128], F32R, tag="w")
                    for g in range(4):
                        nc.sync.dma_start(w_sb[:, :, g, :], w_r[:, :, g, t, :])
                    cp_sb = c_pool.tile([128, BL], F32, tag="cprev")
                    nc.sync.dma_start(cp_sb[:], cT[t * 128:(t + 1) * 128, :])
                    oc_sb = o_pool.tile([128, BL], F32, tag="oc")
                    oh_sb = o_pool.tile([128, BL], F32, tag="oh")

                    for bc in range(NBC):
                        bsl = slice(bc * BCW, (bc + 1) * BCW)
                        ps = []
                        for g in range(4):
                            p_t = psum_pool.tile([128, BCW], F32, tag=f"ps{g}")
                            for c in range(KC):
                                nc.tensor.matmul(
                                    p_t[:], w_sb[:, c, g, :], a_sb[:, c, bsl],
                                    start=(c == 0), stop=(c == KC - 1),
                                )
                            ps.append(p_t)

                        si = act_pool.tile([128, BCW], F32, tag="si")
                        sf = act_pool.tile([128, BCW], F32, tag="sf")
                        so = act_pool.tile([128, BCW], F32, tag="so")
                        tg = act_pool.tile([128, BCW], F32, tag="tg")
                        nc.scalar.activation(si[:], ps[0][:], AF.Sigmoid,
                                             bias=bias_sb[:, 0 * HT + t:0 * HT + t + 1])
                        nc.scalar.activation(sf[:], ps[1][:], AF.Sigmoid,
                                             bias=bias_sb[:, 1 * HT + t:1 * HT + t + 1])
                        nc.scalar.activation(so[:], ps[2][:], AF.Sigmoid,
                                             bias=bias_sb[:, 2 * HT + t:2 * HT + t + 1])
                        nc.scalar.activation(tg[:], ps[3][:], AF.Tanh,
                                             bias=bias_sb[:, 3 * HT + t:3 * HT + t + 1])

                        t1 = act_pool.tile([128, BCW], F32, tag="t1")
                        t2 = act_pool.tile([128, BCW], F32, tag="t2")
                        nc.vector.tensor_tensor(t1[:], sf[:], cp_sb[:, bsl], OP.mult)
                        nc.vector.tensor_tensor(t2[:], si[:], tg[:], OP.mult)
                        nc.vector.tensor_tensor(oc_sb[:, bsl], t1[:], t2[:], OP.add)
                        tct = act_pool.tile([128, BCW], F32, tag="tct")
                        nc.scalar.activation(tct[:], oc_sb[:, bsl], AF.Tanh)
                        nc.vector.tensor_tensor(oh_sb[:, bsl], so[:], tct[:], OP.mult)

                    nc.sync.dma_start(cO[t * 128:(t + 1) * 128, :], oc_sb[:])
                    nc.sync.dma_start(hO[t * 128:(t + 1) * 128, :], oh_sb[:])

            if hw_loop and reps > 1:
                with tc.For_i(0, reps, 1,
                              hint_engines=(mybir.EngineType.PE,),
                              staggered_reset=True):
                    body()
            else:
                for _ in range(reps):
                    body()

    nc.finalize()
    return nc


def kernel(x_current, c_previous, h_previous, Wx, bx, Wh, bh):
    x = np.asarray(x_current, dtype=np.float32)
    c = np.asarray(c_previous, dtype=np.float32)
    h = np.asarray(h_previous, dtype=np.float32)
    Wx = np.asarray(Wx, dtype=np.float32)
    Wh = np.asarray(Wh, dtype=np.float32)
    bsum = np.asarray(bx, dtype=np.float32) + np.asarray(bh, dtype=np.float32)

    wT = np.ascontiguousarray(
        np.concatenate([Wx, Wh], axis=1).T)          # [2048, 4096]
    bias_t = np.ascontiguousarray(bsum.reshape(4 * HT, 128).T)  # [128, 32]

    in_maps = []
    for core in range(NCORES):
        sl = slice(core * BL, (core + 1) * BL)
        aT = np.ascontiguousarray(
            np.concatenate([x[sl], h[sl]], axis=1).T)  # [2048, BL]
        in_maps.append({
            "a_t": aT,
            "w_t": wT,
            "c_t": np.ascontiguousarray(c[sl].T),
            "bias": bias_t,
        })

    if "nc" not in _CACHE:
        _CACHE["nc"] = _build()
    nc = _CACHE["nc"]

    res = run_bass_kernel_spmd(
        nc, in_maps, list(range(NCORES)),
        trace=bool(int(os.environ.get("LSTM_TRACE", "0"))),
    )
    _CACHE["last_result"] = res

    c_out = np.empty((B, H), dtype=np.float32)
    h_out = np.empty((B, H), dtype=np.float32)
    for core in range(NCORES):
        sl = slice(core * BL, (core + 1) * BL)
        c_out[sl] = res.results[core]["c_out"].T
        h_out[sl] = res.results[core]["h_out"].T
    return c_out, h_out



# revision 2
# speedup vs baseline: 1.1237x; 1.1237x over previous
"""Fused LSTM cell on 8 Trainium2 NeuronCores.

Data-parallel over the batch: each core handles 1024 of the 8192 rows.
Per core, the two GEMMs (x @ Wx.T + h @ Wh.T) are fused into one
[2048]-contraction GEMM, with the gate nonlinearities + state update
fused into the PSUM eviction path.

Key performance choices (vs the earlier version of this kernel):
- Host pre-packs activations and weights into the exact SBUF layout,
  so every DMA is a few large contiguous runs per partition instead of
  thousands of 512B descriptors.
- Startup is chunked: the first weight tile and the activation chunks
  stream in k-chunk granularity so the first matmul issues ~3us into
  the kernel instead of waiting ~33us for one monolithic DMA.
- DMAs are spread over three queues: weights on the SP (sync) HWDGE,
  activations/bias/c on the Activation HWDGE, outputs on the Pool
  SWDGE, so big weight transfers never head-of-line-block the small
  latency-critical ones.
- Matmul loop is ordered (gate, kchunk, batch-chunk) so consecutive
  matmuls share the same stationary tile (one LDWEIGHTS can feed both
  batch chunks), and all 8 PSUM banks hold the 4 gates x 2 batch
  chunks of one h-tile.
- Matmul operands are bf16 (fp32r optional via LSTM_VARIANT=f32r):
  same PE rate, half the DMA bytes; accumulation stays fp32 in PSUM.
"""

import os

import ml_dtypes
import numpy as np

import concourse.bacc as bacc
import concourse.mybir as mybir
import concourse.tile as tile
from concourse.bass_utils import run_bass_kernel_spmd

B, I, H = 8192, 1024, 1024
NCORES = 8
BL = B // NCORES        # batch rows per core
G4 = 4 * H              # stacked gate dim
KC = (I + H) // 128     # contraction chunks of 128
HT = H // 128           # h-tiles per core
NBC = 2                 # batch chunks per h-tile
BCW = BL // NBC         # 512 columns per matmul (one PSUM bank)

F32 = mybir.dt.float32
BF16 = ml_dtypes.bfloat16
AF = mybir.ActivationFunctionType
OP = mybir.AluOpType

_CACHE: dict = {}


def _build(variant="bf16"):
    mm_dt = mybir.dt.float32r if variant == "f32r" else mybir.dt.bfloat16
    nc = bacc.Bacc("TRN2", target_bir_lowering=False, debug=False)
    aT = nc.dram_tensor("a_t", [128, KC * BL], mm_dt, kind="ExternalInput")
    wT = nc.dram_tensor("w_t", [128, HT * KC * 4 * 128], mm_dt,
                        kind="ExternalInput")
    cT = nc.dram_tensor("c_t", [H, BL], F32, kind="ExternalInput")
    bias = nc.dram_tensor("bias", [128, 4 * HT], F32, kind="ExternalInput")
    cO = nc.dram_tensor("c_out", [H, BL], F32, kind="ExternalOutput")
    hO = nc.dram_tensor("h_out", [H, BL], F32, kind="ExternalOutput")

    a_view = aT.rearrange("p (c b) -> p c b", c=KC)
    w_view = wT.rearrange("p (t c g j) -> p t c g j", t=HT, c=KC, g=4)

    with tile.TileContext(nc) as tc:
        with (
            tc.tile_pool(name="resident", bufs=1) as res_pool,
            tc.tile_pool(name="wpool", bufs=2) as w_pool,
            tc.tile_pool(name="cpool", bufs=2) as c_pool,
            tc.tile_pool(name="opool", bufs=2) as o_pool,
            tc.tile_pool(name="act", bufs=3) as act_pool,
            tc.tile_pool(name="psum", bufs=1, space="PSUM") as psum_pool,
        ):
            bias_sb = res_pool.tile([128, 4 * HT], F32)
            nc.scalar.dma_start(bias_sb[:], bias[:])
            # Activations resident for the whole kernel, streamed in
            # k-chunks so the first matmul can start almost immediately.
            a_sb = res_pool.tile([128, KC, BL], mm_dt)
            for c in range(KC):
                nc.scalar.dma_start(a_sb[:, c, :], a_view[:, c, :])

            for t in range(HT):
                w_sb = w_pool.tile([128, KC, 4, 128], mm_dt, tag="w")
                if t == 0:
                    # chunked so chunk 0 lands fast
                    for cg in range(4):
                        csl = slice(cg * 4, (cg + 1) * 4)
                        nc.sync.dma_start(w_sb[:, csl], w_view[:, 0, csl])
                else:
                    nc.sync.dma_start(w_sb[:], w_view[:, t])
                cp_sb = c_pool.tile([128, BL], F32, tag="cprev")
                nc.scalar.dma_start(cp_sb[:], cT[t * 128:(t + 1) * 128, :])
                oc_sb = o_pool.tile([128, BL], F32, tag="oc")
                oh_sb = o_pool.tile([128, BL], F32, tag="oh")

                ps = {}
                for g in range(4):
                    for bc in range(NBC):
                        ps[g, bc] = psum_pool.tile(
                            [128, BCW], F32, tag=f"ps{g}{bc}",
                            name=f"ps{g}{bc}", bufs=1)
                # (g, c, bc): the two bc matmuls share one stationary tile
                for g in range(4):
                    for c in range(KC):
                        for bc in range(NBC):
                            bsl = slice(bc * BCW, (bc + 1) * BCW)
                            nc.tensor.matmul(
                                ps[g, bc][:], w_sb[:, c, g, :],
                                a_sb[:, c, bsl],
                                start=(c == 0), stop=(c == KC - 1),
                            )

                for bc in range(NBC):
                    bsl = slice(bc * BCW, (bc + 1) * BCW)
                    si = act_pool.tile([128, BCW], F32, tag="si")
                    sf = act_pool.tile([128, BCW], F32, tag="sf")
                    so = act_pool.tile([128, BCW], F32, tag="so")
                    tg = act_pool.tile([128, BCW], F32, tag="tg")
                    nc.scalar.activation(si[:], ps[0, bc][:], AF.Sigmoid,
                                         bias=bias_sb[:, 0 * HT + t:0 * HT + t + 1])
                    nc.scalar.activation(sf[:], ps[1, bc][:], AF.Sigmoid,
                                         bias=bias_sb[:, 1 * HT + t:1 * HT + t + 1])
                    nc.scalar.activation(so[:], ps[2, bc][:], AF.Sigmoid,
                                         bias=bias_sb[:, 2 * HT + t:2 * HT + t + 1])
                    nc.scalar.activation(tg[:], ps[3, bc][:], AF.Tanh,
                                         bias=bias_sb[:, 3 * HT + t:3 * HT + t + 1])
                    t1 = act_pool.tile([128, BCW], F32, tag="t1")
                    t2 = act_pool.tile([128, BCW], F32, tag="t2")
                    nc.vector.tensor_tensor(t1[:], sf[:], cp_sb[:, bsl], OP.mult)
                    nc.vector.tensor_tensor(t2[:], si[:], tg[:], OP.mult)
                    nc.vector.tensor_tensor(oc_sb[:, bsl], t1[:], t2[:], OP.add)
                    tct = act_pool.tile([128, BCW], F32, tag="tct")
                    nc.scalar.activation(tct[:], oc_sb[:, bsl], AF.Tanh)
                    nc.vector.tensor_tensor(oh_sb[:, bsl], so[:], tct[:], OP.mult)
                    nc.gpsimd.dma_start(cO[t * 128:(t + 1) * 128, bsl],
                                        oc_sb[:, bsl])
                    nc.gpsimd.dma_start(hO[t * 128:(t + 1) * 128, bsl],
                                        oh_sb[:, bsl])

    nc.finalize()
    return nc


def _variant() -> str:
    return os.environ.get("LSTM_VARIANT", "bf16")


def kernel(x_current, c_previous, h_previous, Wx, bx, Wh, bh):
    variant = _variant()
    x = np.asarray(x_current, dtype=np.float32)
    c = np.asarray(c_previous, dtype=np.float32)
    h = np.asarray(h_previous, dtype=np.float32)
    Wx = np.asarray(Wx, dtype=np.float32)
    Wh = np.asarray(Wh, dtype=np.float32)
    bsum = np.asarray(bx, dtype=np.float32) + np.asarray(bh, dtype=np.float32)

    mm_np = np.float32 if variant == "f32r" else BF16

    # [4H, I+H] -> [p, t, c, g, j] so each SBUF weight tile is a single
    # contiguous 16KB-per-partition DMA
    W = np.concatenate([Wx, Wh], axis=1)
    w_prep = np.ascontiguousarray(
        W.reshape(4, HT, 128, KC, 128).transpose(4, 1, 3, 0, 2)
    ).astype(mm_np).reshape(128, HT * KC * 4 * 128)
    bias_t = np.ascontiguousarray(bsum.reshape(4 * HT, 128).T)  # [128, 32]

    in_maps = []
    for core in range(NCORES):
        sl = slice(core * BL, (core + 1) * BL)
        A = np.concatenate([x[sl], h[sl]], axis=1)  # [BL, 2048]
        a_prep = np.ascontiguousarray(
            A.reshape(BL, KC, 128).transpose(2, 1, 0)
        ).astype(mm_np).reshape(128, KC * BL)
        in_maps.append({
            "a_t": a_prep,
            "w_t": w_prep,
            "c_t": np.ascontiguousarray(c[sl].T),
            "bias": bias_t,
        })

    key = f"nc_{variant}"
    if key not in _CACHE:
        _CACHE[key] = _build(variant)
    nc = _CACHE[key]

    res = run_bass_kernel_spmd(
        nc, in_maps, list(range(NCORES)),
        trace=bool(int(os.environ.get("LSTM_TRACE", "0"))),
    )
    _CACHE["last_result"] = res

    c_out = np.empty((B, H), dtype=np.float32)
    h_out = np.empty((B, H), dtype=np.float32)
    for core in range(NCORES):
        sl = slice(core * BL, (core + 1) * BL)
        c_out[sl] = res.results[core]["c_out"].T
        h_out[sl] = res.results[core]["h_out"].T
    return c_out, h_out


# revision 5
# speedup vs baseline: 1.1526x; 1.0257x over previous
"""Fused LSTM cell on 8 Trainium2 NeuronCores.

Data-parallel over the batch: each core handles 1024 of the 8192 rows.
Per core, the two GEMMs (x @ Wx.T + h @ Wh.T) are fused into one
[2048]-contraction GEMM, with the gate nonlinearities + state update
fused into the PSUM eviction path.

Performance structure:
- Host pre-packs activations and weights into the exact SBUF layout,
  so every DMA is a few large contiguous runs per partition instead of
  thousands of 512B descriptors.
- Startup is chunked: tile 0's weights are gate-major so the first
  gate's 512KB lands first on the SP queue, then the activation
  k-chunk pairs stream split across both HWDGE queues; the PE starts
  right after the framework preamble and is only briefly paced by the
  `a` stream during the first gate sweep.
- The SP (sync) queue carries nothing but weights after startup, so
  each tile's 2MB weight prefetch is never stuck behind an output DMA
  waiting on an epilogue semaphore. All small/latency DMAs (bias, c,
  outputs) ride the Activation queue.
- Gate order per tile is [tanh-gate, i, f, o] and the epilogue is
  emitted per-gate as its PSUM banks complete, so the post-last-matmul
  critical chain is just sigmoid(o) -> mult -> DMA instead of the
  whole 7-op gate chain. No SWDGE, which keeps the final drain short.
- Matmul operands are bf16 (fp32r optional via LSTM_VARIANT=f32r):
  same PE rate, half the DMA bytes; accumulation stays fp32 in PSUM.
  Steady-state matmul cadence measured at 216ns per 512-row matmul
  (~99% of the 2.4GHz PE peak), with LDWEIGHTS fully hidden.
"""

import os

import ml_dtypes
import numpy as np

import concourse.bacc as bacc
import concourse.mybir as mybir
import concourse.tile as tile
from concourse.bass_utils import run_bass_kernel_spmd

B, I, H = 8192, 1024, 1024
NCORES = 8
BL = B // NCORES        # batch rows per core
G4 = 4 * H              # stacked gate dim
KC = (I + H) // 128     # contraction chunks of 128
HT = H // 128           # h-tiles per core
NBC = 2                 # batch chunks per h-tile
BCW = BL // NBC         # 512 columns per matmul (one PSUM bank)

F32 = mybir.dt.float32
BF16 = ml_dtypes.bfloat16
AF = mybir.ActivationFunctionType
OP = mybir.AluOpType

# gate completion order: tanh gate (3) first so its tanh read starts
# early; output gate (2) last so only sigmoid(o)*tanh(c) trails the
# final matmul.
GORDER = (3, 0, 1, 2)

_CACHE: dict = {}


def _build(variant="bf16"):
    mm_dt = mybir.dt.float32r if variant == "f32r" else mybir.dt.bfloat16
    nc = bacc.Bacc("TRN2", target_bir_lowering=False, debug=False)
    aT = nc.dram_tensor("a_t", [128, KC * BL], mm_dt, kind="ExternalInput")
    wT = nc.dram_tensor("w_t", [128, HT * KC * 4 * 128], mm_dt,
                        kind="ExternalInput")
    cT = nc.dram_tensor("c_t", [H, BL], F32, kind="ExternalInput")
    bias = nc.dram_tensor("bias", [128, 4 * HT], F32, kind="ExternalInput")
    cO = nc.dram_tensor("c_out", [H, BL], F32, kind="ExternalOutput")
    hO = nc.dram_tensor("h_out", [H, BL], F32, kind="ExternalOutput")

    a_view = aT.rearrange("p (c b) -> p c b", c=KC)
    # tiles 1..7 are [c, g, j] per partition; tile 0's block is packed
    # gate-major [g, c, j] so per-gate startup DMAs are contiguous
    w_view = wT.rearrange("p (t c g j) -> p t c g j", t=HT, c=KC, g=4)
    w0_view = wT.rearrange("p (t g c j) -> p t g c j", t=HT, g=4, c=KC)

    with tile.TileContext(nc) as tc:
        with (
            tc.tile_pool(name="resident", bufs=1) as res_pool,
            tc.tile_pool(name="wpool", bufs=2) as w_pool,
            tc.tile_pool(name="cpool", bufs=2) as c_pool,
            tc.tile_pool(name="opool", bufs=2) as o_pool,
            tc.tile_pool(name="act", bufs=3) as act_pool,
            tc.tile_pool(name="psum", bufs=1, space="PSUM") as psum_pool,
        ):
            # ---- startup choreography -------------------------------
            # SP queue:      w0[g3] | a odd pairs | w0[g0,g1,g2] | w1..w7
            # Act queue:     bias | a even pairs | c0 | per-tile c/outs
            w0_sb = w_pool.tile([128, 4, KC, 128], mm_dt, tag="w0", bufs=1)
            nc.sync.dma_start(w0_sb[:, GORDER[0]], w0_view[:, 0, GORDER[0]])

            bias_sb = res_pool.tile([128, 4 * HT], F32)
            nc.scalar.dma_start(bias_sb[:], bias[:])
            a_sb = res_pool.tile([128, KC, BL], mm_dt)
            for cp in range(KC // 2):
                csl = slice(2 * cp, 2 * cp + 2)
                eng = nc.scalar if cp % 2 == 0 else nc.sync
                eng.dma_start(a_sb[:, csl], a_view[:, csl])
            for gi in GORDER[1:]:
                nc.sync.dma_start(w0_sb[:, gi], w0_view[:, 0, gi])

            for t in range(HT):
                if t == 0:
                    def w_ap(c, g, _w=w0_sb):
                        return _w[:, g, c, :]
                else:
                    w_sb = w_pool.tile([128, KC, 4, 128], mm_dt, tag="w")
                    nc.sync.dma_start(w_sb[:], w_view[:, t])

                    def w_ap(c, g, _w=w_sb):
                        return _w[:, c, g, :]

                cp_sb = c_pool.tile([128, BL], F32, tag="cprev")
                nc.scalar.dma_start(cp_sb[:], cT[t * 128:(t + 1) * 128, :])
                oc_sb = o_pool.tile([128, BL], F32, tag="oc")
                oh_sb = o_pool.tile([128, BL], F32, tag="oh")

                ps = {}
                for g in range(4):
                    for bc in range(NBC):
                        ps[g, bc] = psum_pool.tile(
                            [128, BCW], F32, tag=f"ps{g}{bc}",
                            name=f"ps{g}{bc}", bufs=1)
                ep = {}
                for bc in range(NBC):
                    for nm in ("si", "sf", "so", "tg", "t1", "t2", "tct"):
                        ep[nm, bc] = act_pool.tile([128, BCW], F32,
                                                   tag=f"{nm}{bc}",
                                                   name=f"{nm}{bc}")

                def bias_ap(g):
                    return bias_sb[:, g * HT + t:g * HT + t + 1]

                for g in GORDER:
                    # (c, bc) ordering: both bc matmuls share a stationary
                    for c in range(KC):
                        for bc in range(NBC):
                            bsl = slice(bc * BCW, (bc + 1) * BCW)
                            nc.tensor.matmul(
                                ps[g, bc][:], w_ap(c, g), a_sb[:, c, bsl],
                                start=(c == 0), stop=(c == KC - 1),
                            )
                    # emit the epilogue ops that become ready once this
                    # gate's banks stop — they overlap the next gates'
                    # matmuls and release PSUM banks early
                    for bc in range(NBC):
                        bsl = slice(bc * BCW, (bc + 1) * BCW)
                        if g == 3:
                            nc.scalar.activation(ep["tg", bc][:], ps[3, bc][:],
                                                 AF.Tanh, bias=bias_ap(3))
                        elif g == 0:
                            nc.scalar.activation(ep["si", bc][:], ps[0, bc][:],
                                                 AF.Sigmoid, bias=bias_ap(0))
                            nc.vector.tensor_tensor(
                                ep["t2", bc][:], ep["si", bc][:],
                                ep["tg", bc][:], OP.mult)
                        elif g == 1:
                            nc.scalar.activation(ep["sf", bc][:], ps[1, bc][:],
                                                 AF.Sigmoid, bias=bias_ap(1))
                            nc.vector.tensor_tensor(
                                ep["t1", bc][:], ep["sf", bc][:],
                                cp_sb[:, bsl], OP.mult)
                            nc.vector.tensor_tensor(
                                oc_sb[:, bsl], ep["t1", bc][:],
                                ep["t2", bc][:], OP.add)
                            nc.scalar.activation(ep["tct", bc][:],
                                                 oc_sb[:, bsl], AF.Tanh)
                            nc.scalar.dma_start(
                                cO[t * 128:(t + 1) * 128, bsl],
                                oc_sb[:, bsl])
                        else:  # g == 2
                            nc.scalar.activation(ep["so", bc][:], ps[2, bc][:],
                                                 AF.Sigmoid, bias=bias_ap(2))
                            nc.vector.tensor_tensor(
                                oh_sb[:, bsl], ep["so", bc][:],
                                ep["tct", bc][:], OP.mult)
                            nc.scalar.dma_start(
                                hO[t * 128:(t + 1) * 128, bsl],
                                oh_sb[:, bsl])

    nc.finalize()
    return nc


def _variant() -> str:
    return os.environ.get("LSTM_VARIANT", "bf16")


def kernel(x_current, c_previous, h_previous, Wx, bx, Wh, bh):
    variant = _variant()
    x = np.asarray(x_current, dtype=np.float32)
    c = np.asarray(c_previous, dtype=np.float32)
    h = np.asarray(h_previous, dtype=np.float32)
    Wx = np.asarray(Wx, dtype=np.float32)
    Wh = np.asarray(Wh, dtype=np.float32)
    bsum = np.asarray(bx, dtype=np.float32) + np.asarray(bh, dtype=np.float32)

    mm_np = np.float32 if variant == "f32r" else BF16

    # [4H, I+H] -> per-tile SBUF blocks: tile 0 gate-major [g, c, j],
    # tiles 1..7 chunk-major [c, g, j]; contiguous 16KB/partition DMAs
    W = np.concatenate([Wx, Wh], axis=1)
    w5 = W.reshape(4, HT, 128, KC, 128).transpose(4, 1, 3, 0, 2)  # p t c g j
    w_prep = np.ascontiguousarray(w5).astype(mm_np)
    w_prep[:, 0] = np.ascontiguousarray(
        w5[:, 0].transpose(0, 2, 1, 3)          # p g c j for tile 0
    ).astype(mm_np).reshape(128, KC, 4, 128)
    w_prep = w_prep.reshape(128, HT * KC * 4 * 128)
    bias_t = np.ascontiguousarray(bsum.reshape(4 * HT, 128).T)  # [128, 32]

    in_maps = []
    for core in range(NCORES):
        sl = slice(core * BL, (core + 1) * BL)
        A = np.concatenate([x[sl], h[sl]], axis=1)  # [BL, 2048]
        a_prep = np.ascontiguousarray(
            A.reshape(BL, KC, 128).transpose(2, 1, 0)
        ).astype(mm_np).reshape(128, KC * BL)
        in_maps.append({
            "a_t": a_prep,
            "w_t": w_prep,
            "c_t": np.ascontiguousarray(c[sl].T),
            "bias": bias_t,
        })

    key = f"nc_{variant}"
    if key not in _CACHE:
        _CACHE[key] = _build(variant)
    nc = _CACHE[key]

    res = run_bass_kernel_spmd(
        nc, in_maps, list(range(NCORES)),
        trace=bool(int(os.environ.get("LSTM_TRACE", "0"))),
    )
    _CACHE["last_result"] = res

    c_out = np.empty((B, H), dtype=np.float32)
    h_out = np.empty((B, H), dtype=np.float32)
    for core in range(NCORES):
        sl = slice(core * BL, (core + 1) * BL)
        c_out[sl] = res.results[core]["c_out"].T
        h_out[sl] = res.results[core]["h_out"].T
    return c_out, h_out


# revision 11
# speedup vs baseline: 1.1707x; 1.0157x over previous
"""Fused LSTM cell on 8 Trainium2 NeuronCores.

Data-parallel over the batch: each core handles 1024 of the 8192 rows.
Per core, the two GEMMs (x @ Wx.T + h @ Wh.T) are fused into one
[2048]-contraction GEMM, with the gate nonlinearities + state update
fused into the PSUM eviction path.

Performance structure:
- Host pre-packs activations and weights into the exact SBUF layout,
  so every DMA is a few large contiguous runs per partition instead of
  thousands of 512B descriptors.
- Startup is chunked: tile 0's weights are gate-major so the first
  gate's 512KB lands first on the SP queue, then the activation
  k-chunk pairs stream split across both HWDGE queues; the PE starts
  right after the framework preamble and is only briefly paced by the
  `a` stream during the first gate sweep.
- The SP (sync) queue carries nothing but weights after startup, so
  each tile's 2MB weight prefetch is never stuck behind an output DMA
  waiting on an epilogue semaphore. All small/latency DMAs (bias, c,
  outputs) ride the Activation queue.
- Gate order per tile is [tanh-gate, i, f, o] and the epilogue is
  emitted per-gate as its PSUM banks complete, so the post-last-matmul
  critical chain is just sigmoid(o) -> mult -> DMA instead of the
  whole 7-op gate chain. No SWDGE, which keeps the final drain short.
- Matmul operands are bf16 (fp32r optional via LSTM_VARIANT=f32r):
  same PE rate, half the DMA bytes; accumulation stays fp32 in PSUM.
  Steady-state matmul cadence measured at 216ns per 512-row matmul
  (~99% of the 2.4GHz PE peak), with LDWEIGHTS fully hidden.
"""

import os

import ml_dtypes
import numpy as np

import concourse.bacc as bacc
import concourse.mybir as mybir
import concourse.tile as tile
from concourse.bass_utils import run_bass_kernel_spmd

B, I, H = 8192, 1024, 1024
NCORES = 8
BL = B // NCORES        # batch rows per core
G4 = 4 * H              # stacked gate dim
KC = (I + H) // 128     # contraction chunks of 128
HT = H // 128           # h-tiles per core
NBC = 2                 # batch chunks per h-tile
BCW = BL // NBC         # 512 columns per matmul (one PSUM bank)

F32 = mybir.dt.float32
BF16 = ml_dtypes.bfloat16
AF = mybir.ActivationFunctionType
OP = mybir.AluOpType

# gate completion order: tanh gate (3) first so its tanh read starts
# early; output gate (2) last so only sigmoid(o)*tanh(c) trails the
# final matmul.
GORDER = (3, 0, 1, 2)

_CACHE: dict = {}


def _build(variant="bf16"):
    mm_dt = mybir.dt.float32r if variant == "f32r" else mybir.dt.bfloat16
    nc = bacc.Bacc("TRN2", target_bir_lowering=False, debug=False)
    aT = nc.dram_tensor("a_t", [128, KC * BL], mm_dt, kind="ExternalInput")
    wT = nc.dram_tensor("w_t", [128, HT * KC * 4 * 128], mm_dt,
                        kind="ExternalInput")
    cT = nc.dram_tensor("c_t", [H, BL], F32, kind="ExternalInput")
    bias = nc.dram_tensor("bias", [128, 4 * HT], F32, kind="ExternalInput")
    cO = nc.dram_tensor("c_out", [H, BL], F32, kind="ExternalOutput")
    hO = nc.dram_tensor("h_out", [H, BL], F32, kind="ExternalOutput")

    a_view = aT.rearrange("p (c b) -> p c b", c=KC)
    w_view = wT.rearrange("p (t c g j) -> p t c g j", t=HT, c=KC, g=4)

    with tile.TileContext(nc) as tc:
        with (
            tc.tile_pool(name="resident", bufs=1) as res_pool,
            tc.tile_pool(name="wpool", bufs=2) as w_pool,
            tc.tile_pool(name="cpool", bufs=2) as c_pool,
            tc.tile_pool(name="opool", bufs=2) as o_pool,
            tc.tile_pool(name="act", bufs=3) as act_pool,
            tc.tile_pool(name="psum", bufs=1, space="PSUM") as psum_pool,
        ):
            # ---- startup choreography -------------------------------
            # SP queue:   w0 c-group | a pair | w0 c-group | ... | w1..w7
            # Act queue:  bias | a even pairs | c0 | per-tile c/outs
            # Tile 0 runs its matmuls k-chunk-outer so each arriving
            # a-chunk is consumed by 8 matmuls immediately; the PE then
            # outpaces neither queue and never stalls on the a stream.
            w0_sb = w_pool.tile([128, KC, 4, 128], mm_dt, tag="w0", bufs=1)
            bias_sb = res_pool.tile([128, 4 * HT], F32)
            nc.scalar.dma_start(bias_sb[:], bias[:])
            a_sb = res_pool.tile([128, KC, BL], mm_dt)
            for q in range(4):
                csl = slice(q * 4, (q + 1) * 4)
                nc.sync.dma_start(w0_sb[:, csl], w_view[:, 0, csl])
                lo = slice(q * 4, q * 4 + 2)
                hi = slice(q * 4 + 2, q * 4 + 4)
                nc.sync.dma_start(a_sb[:, lo], a_view[:, lo])
                nc.scalar.dma_start(a_sb[:, hi], a_view[:, hi])

            for t in range(HT):
                if t == 0:
                    w_sb = w0_sb
                else:
                    w_sb = w_pool.tile([128, KC, 4, 128], mm_dt, tag="w")
                    nc.sync.dma_start(w_sb[:], w_view[:, t])

                cp_sb = c_pool.tile([128, BL], F32, tag="cprev")
                nc.scalar.dma_start(cp_sb[:], cT[t * 128:(t + 1) * 128, :])
                oc_sb = o_pool.tile([128, BL], F32, tag="oc")
                oh_sb = o_pool.tile([128, BL], F32, tag="oh")

                ps = {}
                for g in range(4):
                    for bc in range(NBC):
                        ps[g, bc] = psum_pool.tile(
                            [128, BCW], F32, tag=f"ps{g}{bc}",
                            name=f"ps{g}{bc}", bufs=1)
                ep = {}
                for bc in range(NBC):
                    for nm in ("si", "sf", "so", "tg", "t1", "t2", "tct"):
                        ep[nm, bc] = act_pool.tile([128, BCW], F32,
                                                   tag=f"{nm}{bc}",
                                                   name=f"{nm}{bc}")

                def bias_ap(g):
                    return bias_sb[:, g * HT + t:g * HT + t + 1]

                if t == 0:
                    # k-chunk-outer: all 8 banks accumulate together so
                    # each a-chunk is consumed as soon as it arrives
                    for c in range(KC):
                        for g in GORDER:
                            for bc in range(NBC):
                                bsl = slice(bc * BCW, (bc + 1) * BCW)
                                nc.tensor.matmul(
                                    ps[g, bc][:], w_sb[:, c, g, :],
                                    a_sb[:, c, bsl],
                                    start=(c == 0), stop=(c == KC - 1),
                                )

                for g in GORDER:
                    if t != 0:
                        # (c, bc) ordering: both bc matmuls share a
                        # stationary tile
                        for c in range(KC):
                            for bc in range(NBC):
                                bsl = slice(bc * BCW, (bc + 1) * BCW)
                                nc.tensor.matmul(
                                    ps[g, bc][:], w_sb[:, c, g, :],
                                    a_sb[:, c, bsl],
                                    start=(c == 0), stop=(c == KC - 1),
                                )
                    # emit the epilogue ops that become ready once this
                    # gate's banks stop — they overlap the next gates'
                    # matmuls and release PSUM banks early
                    for bc in range(NBC):
                        bsl = slice(bc * BCW, (bc + 1) * BCW)
                        if g == 3:
                            nc.scalar.activation(ep["tg", bc][:], ps[3, bc][:],
                                                 AF.Tanh, bias=bias_ap(3))
                        elif g == 0:
                            nc.scalar.activation(ep["si", bc][:], ps[0, bc][:],
                                                 AF.Sigmoid, bias=bias_ap(0))
                            nc.vector.tensor_tensor(
                                ep["t2", bc][:], ep["si", bc][:],
                                ep["tg", bc][:], OP.mult)
                        elif g == 1:
                            nc.scalar.activation(ep["sf", bc][:], ps[1, bc][:],
                                                 AF.Sigmoid, bias=bias_ap(1))
                            nc.vector.tensor_tensor(
                                ep["t1", bc][:], ep["sf", bc][:],
                                cp_sb[:, bsl], OP.mult)
                            nc.vector.tensor_tensor(
                                oc_sb[:, bsl], ep["t1", bc][:],
                                ep["t2", bc][:], OP.add)
                            nc.scalar.activation(ep["tct", bc][:],
                                                 oc_sb[:, bsl], AF.Tanh)
                            nc.scalar.dma_start(
                                cO[t * 128:(t + 1) * 128, bsl],
                                oc_sb[:, bsl])
                        else:  # g == 2
                            nc.scalar.activation(ep["so", bc][:], ps[2, bc][:],
                                                 AF.Sigmoid, bias=bias_ap(2))
                            nc.vector.tensor_tensor(
                                oh_sb[:, bsl], ep["so", bc][:],
                                ep["tct", bc][:], OP.mult)
                            nc.scalar.dma_start(
                                hO[t * 128:(t + 1) * 128, bsl],
                                oh_sb[:, bsl])

    nc.finalize()
    return nc


def _variant() -> str:
    return os.environ.get("LSTM_VARIANT", "bf16")


def kernel(x_current, c_previous, h_previous, Wx, bx, Wh, bh):
    variant = _variant()
    x = np.asarray(x_current, dtype=np.float32)
    c = np.asarray(c_previous, dtype=np.float32)
    h = np.asarray(h_previous, dtype=np.float32)
    Wx = np.asarray(Wx, dtype=np.float32)
    Wh = np.asarray(Wh, dtype=np.float32)
    bsum = np.asarray(bx, dtype=np.float32) + np.asarray(bh, dtype=np.float32)

    mm_np = np.float32 if variant == "f32r" else BF16

    # [4H, I+H] -> per-tile SBUF blocks [c, g, j] per partition;
    # contiguous 16KB-per-partition DMAs
    W = np.concatenate([Wx, Wh], axis=1)
    w5 = W.reshape(4, HT, 128, KC, 128).transpose(4, 1, 3, 0, 2)  # p t c g j
    w_prep = np.ascontiguousarray(w5).astype(mm_np).reshape(
        128, HT * KC * 4 * 128)
    bias_t = np.ascontiguousarray(bsum.reshape(4 * HT, 128).T)  # [128, 32]

    in_maps = []
    for core in range(NCORES):
        sl = slice(core * BL, (core + 1) * BL)
        A = np.concatenate([x[sl], h[sl]], axis=1)  # [BL, 2048]
        a_prep = np.ascontiguousarray(
            A.reshape(BL, KC, 128).transpose(2, 1, 0)
        ).astype(mm_np).reshape(128, KC * BL)
        in_maps.append({
            "a_t": a_prep,
            "w_t": w_prep,
            "c_t": np.ascontiguousarray(c[sl].T),
            "bias": bias_t,
        })

    key = f"nc_{variant}"
    if key not in _CACHE:
        _CACHE[key] = _build(variant)
    nc = _CACHE[key]

    res = run_bass_kernel_spmd(
        nc, in_maps, list(range(NCORES)),
        trace=bool(int(os.environ.get("LSTM_TRACE", "0"))),
    )
    _CACHE["last_result"] = res

    c_out = np.empty((B, H), dtype=np.float32)
    h_out = np.empty((B, H), dtype=np.float32)
    for core in range(NCORES):
        sl = slice(core * BL, (core + 1) * BL)
        c_out[sl] = res.results[core]["c_out"].T
        h_out[sl] = res.results[core]["h_out"].T
    return c_out, h_out
